# revision 66
# baseline (speedup 1.0000x reference)
"""MaxMargin loss kernel for 8 Trainium2 NeuronCores.

Reference computation (B=8192 rows, D=512, S=25 negative rounds):
    cos_pos[b]   = <y_true[b], y_pred[b]> / max(|y_true[b]||y_pred[b]|, eps)
    cos_neg[s,b] = <y_true[perm[s,b]], y_pred[b]> / max(|y_true[perm[s,b]]||y_pred[b]|, eps)
    out = mean_b( sum_s relu(1 - cos_pos + cos_neg) ) / S

Strategy: data-parallel over the batch dim (1024 rows of y_pred per
core); host casts inputs to bf16 (layout/precision prep only).

Prepass (software-pipelined): each core normalizes the full y_true into
an fp8-e4m3 row table in its DRAM.  Rows sit in (p n) layout so every
slab DMA is one contiguous 8KB chunk per partition; squares split
3 DVE / 5 ACT; normalize-copies run on DVE; slab l+1's squares are
emitted before slab l's copies so the rsqrt chain never stalls DVE, and
y_pred's copies are deferred past the last table write to overlap the
first gather's descriptor generation.

Gather phase: one single-packet dma_gather per round (26 rounds x 1024
rows x 512B, incl. the identity round 0 for cos_pos) from the fp8
table, 4 SWDGE queues round-robin.  The gather index order is permuted
host-side so gathered row i pairs with the (p n)-layout y_pred block.
Dots run as fused STT multiply+row-reduce ops on DVE (the measured
phase-3 floor: DVE is busy ~97% of the gather phase).  Margins run as
fused relu+sum activations on ACT; the final cross-partition sum is one
1-column matmul.  Host sums the 8 per-core partials.

Four tiny warmup gathers at prepass start absorb the first-use cost of
the SWDGE gather path while GPSIMD is idle; round 0 is split into four
256-row chunk gathers on separate queues so the first dots start ~6us
after the last table write instead of ~14us (a full 1024-row first gen
has ~9us completion latency); and cos_pos's (1 - cos) bias is computed
right after round 0 so the ACT margin ops overlap the final rounds'
dots.

Measured: ~213.4us on HW (baseline 310.2us, 1.45x), rel err ~1e-5 vs
the f32 reference (harness gate 2e-2).
"""

import os
import sys

import numpy as np

for _p in ("/opt/trn_rl_repo", "/root/.axon_site/_ro/trn_rl_repo"):
    if os.path.isdir(_p) and _p not in sys.path:
        sys.path.append(_p)

import ml_dtypes

import concourse.bacc as bacc
import concourse.bass as bass
import concourse.mybir as mybir
import concourse.tile as tile
from concourse.bass_utils import run_bass_kernel_spmd

B = 8192          # total batch rows
D = 512           # feature dim
S = 25            # negative-sampling rounds
NCORES = 8
SH = B // NCORES  # rows per core (1024)
NB = SH // 128    # 128-row blocks per core (8)
NSLAB = B // SH   # y_true slabs for the normalize pass (8)
NR = S + 1        # gather rounds incl. identity round 0 (26)
RB = 1            # rounds per dma_gather (single-packet)
GBUFS = 4         # gather tile buffering
NQ = 4            # swdge queues; round-robin gathers across them
F32 = mybir.dt.float32
BF16 = mybir.dt.bfloat16
I16 = mybir.dt.int16
FP8 = mybir.dt.float8e4

AX = mybir.AxisListType
ALU = mybir.AluOpType
ACTF = mybir.ActivationFunctionType


def build_program():
    nc = bacc.Bacc(None, target_bir_lowering=False, num_swdge_queues=NQ)

    yp = nc.dram_tensor("yp", [SH, D], BF16, kind="ExternalInput")
    yt = nc.dram_tensor("yt", [B, D], BF16, kind="ExternalInput")
    idx = nc.dram_tensor("idx", [128, NR * 64], I16, kind="ExternalInput")
    tt = nc.dram_tensor("tt", [B, D], BF16, kind="Internal")
    out = nc.dram_tensor("out", [1, 1], F32, kind="ExternalOutput")

    with tile.TileContext(nc) as tc:
        with (
            tc.tile_pool(name="singles", bufs=1) as singles,
            tc.tile_pool(name="slab", bufs=5) as slab_pool,
            tc.tile_pool(name="upool", bufs=5) as u_pool,
            tc.tile_pool(name="gpool", bufs=GBUFS) as g_pool,
            tc.tile_pool(name="scr", bufs=6) as scr_pool,
            tc.tile_pool(name="small", bufs=6) as small_pool,
            tc.tile_pool(name="psum", bufs=1, space="PSUM") as psum_pool,
        ):
            idx_sb = singles.tile([128, NR * 64], I16)

            zero_b = singles.tile([128, 1], F32)
            nc.vector.memset(zero_b, 0.0)

            def dot_stt(in0, in1, accum_out):
                """fused multiply + row-reduce via STT; one DVE pass (1x)."""
                scr = scr_pool.tile([128, D], BF16, tag="dot_scr")
                nc.vector.scalar_tensor_tensor(
                    out=scr,
                    in0=in0,
                    scalar=1.0,
                    in1=in1,
                    op0=ALU.mult,
                    op1=ALU.mult,
                    accum_out=accum_out,
                )

            def dot_act_sq(in0, in1, accum_out):
                """DVE bf16 add (2x) + ACT square-accum on the idle engine.

                accum_out = ||in0 + in1||^2 = 2 + 2*cos for unit rows;
                those cn columns are rescaled to cos before the margins."""
                scr = scr_pool.tile([128, D], BF16, tag="dot_scr")
                nc.vector.tensor_tensor(
                    out=scr, in0=in0, in1=in1, op=ALU.add
                )
                act_scr = scr_pool.tile([128, D], BF16, tag="act_scr")
                nc.scalar.activation(
                    out=act_scr,
                    in_=scr,
                    func=ACTF.Square,
                    bias=0.0,
                    scale=1.0,
                    accum_out=accum_out,
                )

            def square_pass(x):
                """rowwise 1/|row| for a [128, NB, D] bf16 slab -> [128, NB].

                Squares split 3 DVE / 5 ACT (ACT's accumulator read makes
                its squares ~1.4x a DVE STT)."""
                ssq = small_pool.tile([128, NB], F32, tag="ssq")
                for n in range(NB):
                    if n % 3 == 0:
                        dot_stt(x[:, n, :], x[:, n, :], ssq[:, n : n + 1])
                    else:
                        act_scr = scr_pool.tile([128, D], BF16, tag="act_scr")
                        nc.scalar.activation(
                            out=act_scr,
                            in_=x[:, n, :],
                            func=ACTF.Square,
                            bias=0.0,
                            scale=1.0,
                            accum_out=ssq[:, n : n + 1],
                        )
                ssqm = small_pool.tile([128, NB], F32, tag="ssqm")
                nc.vector.tensor_scalar_max(out=ssqm, in0=ssq, scalar1=1e-30)
                inv = small_pool.tile([128, NB], F32, tag="inv")
                nc.vector.reciprocal(out=inv, in_=ssqm)
                rs = small_pool.tile([128, NB], F32, tag="rs")
                nc.scalar.activation(
                    out=rs, in_=inv, func=ACTF.Sqrt, bias=zero_b, scale=1.0
                )
                return rs

            def copy_pass(x, rs, u):
                for n in range(NB):
                    nc.vector.tensor_scalar_mul(
                        out=u[:, n, :], in0=x[:, n, :], scalar1=rs[:, n : n + 1]
                    )

            # ---- software-pipelined prepass ------------------------------
            # Emit slab l+1's squares before slab l's copies so DVE never
            # stalls on the rsqrt chain; y_pred's copies are deferred to the
            # end, overlapping the first gather's descriptor generation.
            up = singles.tile([128, NB, D], BF16)
            xp = singles.tile([128, NB, D], BF16)
            ypv = yp[:, :].rearrange("(p n) d -> p n d", n=NB)
            nc.sync.dma_start(out=xp[:, : NB // 2, :], in_=ypv[:, : NB // 2, :])
            nc.sync.dma_start(out=xp[:, NB // 2 :, :], in_=ypv[:, NB // 2 :, :])
            nc.sync.dma_start(out=idx_sb, in_=idx[:, :])

            # Warm up the SWDGE gather machinery (one-time pool-config/ring
            # init costs ~9us) with tiny 16-row gathers on each queue while
            # GPSIMD is otherwise idle; sources yt, which is ready at t=0.
            ytw = yt[:, :].rearrange("b (c e) -> (b c) e", e=128)
            for q in range(NQ):
                warm = scr_pool.tile([128, 1, 128], BF16, tag="warm")
                nc.gpsimd.dma_gather(
                    warm[:, :, :],
                    ytw,
                    idx_sb[:, 0:1],
                    num_idxs=16,
                    num_idxs_reg=16,
                    elem_size=128,
                    single_packet=True,
                    queue_num=q,
                )
            rs_p = square_pass(xp)
            rs_p_keep = singles.tile([128, NB], F32)
            nc.vector.tensor_copy(out=rs_p_keep, in_=rs_p)

            pend = None  # (x, rs, u, l) awaiting copies+writeback
            for l in range(NSLAB):
                x = slab_pool.tile([128, NB, D], BF16, tag="x")
                nc.sync.dma_start(
                    out=x,
                    in_=yt[l * SH : (l + 1) * SH, :].rearrange(
                        "(p n) d -> p n d", n=NB
                    ),
                )
                rs = square_pass(x)
                if pend is not None:
                    px, prs, pu, pl = pend
                    copy_pass(px, prs, pu)
                    nc.sync.dma_start(
                        out=tt[pl * SH : (pl + 1) * SH, :].rearrange(
                            "(p n) d -> p n d", n=NB
                        ),
                        in_=pu,
                    )
                u = u_pool.tile([128, NB, D], BF16, tag="u")
                pend = (x, rs, u, l)
            px, prs, pu, pl = pend
            ttv = tt[pl * SH : (pl + 1) * SH, :].rearrange(
                "(p n) d -> p n d", n=NB
            )
            h = NB // 2
            for n in range(h):
                nc.vector.tensor_scalar_mul(
                    out=pu[:, n, :], in0=px[:, n, :], scalar1=prs[:, n : n + 1]
                )
            nc.sync.dma_start(out=ttv[:, :h, :], in_=pu[:, :h, :])
            for n in range(h, NB):
                nc.vector.tensor_scalar_mul(
                    out=pu[:, n, :], in0=px[:, n, :], scalar1=prs[:, n : n + 1]
                )
            nc.sync.dma_start(out=ttv[:, h:, :], in_=pu[:, h:, :])

            # late warmup: a dummy gather anchored (via a data dep on the
            # last slab's rsqrt) to run right before the first real gather,
            # absorbing the ~9us first-gen-after-idle SWDGE penalty while
            # the final table writes drain.  All-zero indices gather row 0.
            zf = small_pool.tile([128, 1], F32, tag="zf")
            nc.vector.tensor_scalar_mul(out=zf, in0=prs[:, 0:1], scalar1=0.0)
            idx_dummy = small_pool.tile([128, 1], I16, tag="idxd")
            nc.vector.tensor_copy(out=idx_dummy, in_=zf)
            warm_l = scr_pool.tile([128, 1, 128], BF16, tag="warm")
            nc.gpsimd.dma_gather(
                warm_l[:, :, :],
                ytw,
                idx_dummy[:, 0:1],
                num_idxs=16,
                num_idxs_reg=16,
                elem_size=128,
                single_packet=True,
                queue_num=0,
            )
            copy_pass(xp, rs_p_keep, up)

            # ---- gather rounds + fused dot products ----
            # CN[:, n, r] = cos of round r for row block n (round 0 = cos_pos)
            # rounds >= V3_START hold 2 + 2*cos instead (variant 3).
            cn = singles.tile([128, NB, NR], F32)
            nc.vector.memset(cn, 0.0)
            cpb = singles.tile([128, NB], F32)  # 1 - cos_pos
            # Round 0 is split into 4 small gathers on separate queues: the
            # first 1024-row gen after the prepass costs ~8.6us (cold SWDGE
            # path), gating phase-3 start; 256-row chunks start the dots
            # several us earlier and per-chunk tiles keep the deps precise.
            for c in range(4):
                gc = scr_pool.tile([128, 2, D], BF16, tag="g0")
                nc.gpsimd.dma_gather(
                    gc[:, :, :],
                    tt[:, :],
                    idx_sb[:, c * 16 : (c + 1) * 16],
                    num_idxs=SH // 4,
                    num_idxs_reg=SH // 4,
                    elem_size=D,
                    single_packet=True,
                    queue_num=c,
                )
                for k in range(2):
                    n = 2 * c + k
                    dot_stt(gc[:, k, :], up[:, n, :], cn[:, n, 0:1])
            for s in range(1, NR):
                g = g_pool.tile([128, NB, D], BF16, tag="g")
                nc.gpsimd.dma_gather(
                    g[:, :, :],
                    tt[:, :],
                    idx_sb[:, s * 64 : (s + 1) * 64],
                    num_idxs=SH,
                    num_idxs_reg=SH,
                    elem_size=D,
                    single_packet=(SH // 16) <= 64,
                    queue_num=s % NQ,
                )
                for n in range(NB):
                    dot_stt(g[:, n, :], up[:, n, :], cn[:, n, s : s + 1])
                if s == 1:
                    # cpb needs only round 0; computing it here lets the
                    # ACT margin ops overlap the final rounds' dots.
                    nc.vector.tensor_scalar(
                        out=cpb,
                        in0=cn[:, :, 0],
                        scalar1=-1.0,
                        scalar2=1.0,
                        op0=ALU.mult,
                        op1=ALU.add,
                    )

            # ---- margins: sum_s relu((1 - cos_pos) + cos_neg) ----
            # margin+sum fused on ACT: mt[:, n] = sum_s relu(cn + cpb)
            mt = singles.tile([128, NB], F32)
            for n in range(NB):
                m_scr = scr_pool.tile([128, S], F32, tag="m_scr")
                nc.scalar.activation(
                    out=m_scr,
                    in_=cn[:, n, 1:NR],
                    func=ACTF.Relu,
                    bias=cpb[:, n : n + 1],
                    scale=1.0,
                    accum_out=mt[:, n : n + 1],
                )

            # ---- partial = sum over partitions and blocks ----
            mts = singles.tile([128, 1], F32)
            nc.vector.reduce_sum(out=mts, in_=mt, axis=AX.X)
            ones = singles.tile([128, 1], F32)
            nc.vector.memset(ones, 1.0)
            ps = psum_pool.tile([1, 1], F32)
            nc.tensor.matmul(ps, ones, mts, start=True, stop=True)
            osb = singles.tile([1, 1], F32)
            nc.vector.tensor_copy(out=osb, in_=ps)
            nc.sync.dma_start(out=out[:, :], in_=osb)

    return nc


def make_in_maps(y_pred, y_true, perm):
    """Shard the full inputs into the 8 per-core input maps."""
    y_pred = np.ascontiguousarray(y_pred, dtype=np.float32).astype(
        ml_dtypes.bfloat16
    )
    y_true = np.ascontiguousarray(y_true, dtype=np.float32).astype(
        ml_dtypes.bfloat16
    )
    perm = np.asarray(perm)
    in_maps = []
    for c in range(NCORES):
        ident = (c * SH + np.arange(SH, dtype=np.int64))[None, :]
        rounds = np.concatenate(
            [ident, perm[:, c * SH : (c + 1) * SH].astype(np.int64)], axis=0
        )  # [NR, SH]
        # dma_gather index layout: flat index i lives at partition i%16,
        # free slot i//16, replicated across the 8 groups of 16 partitions.
        # g row at gather position i lands at [i%128, i//128]; up (p n)
        # layout puts batch row b at [b//NB, b%NB] -> remap i = (b%NB)*128+b//NB
        i_of = np.arange(SH)
        remap = (i_of % 128) * NB + i_of // 128  # b gathered at position i
        rounds = rounds[:, remap]
        w = rounds.reshape(NR, SH // 16, 16).transpose(0, 2, 1)  # [NR,16,64]
        rep = np.broadcast_to(w[:, None, :, :], (NR, 8, 16, SH // 16))
        idx = (
            rep.reshape(NR, 128, SH // 16)
            .transpose(1, 0, 2)
            .reshape(128, NR * (SH // 16))
            .astype(np.int16)
        )
        in_maps.append(
            {
                "yp": np.ascontiguousarray(y_pred[c * SH : (c + 1) * SH]),
                "yt": y_true,
                "idx": np.ascontiguousarray(idx),
            }
        )
    return in_maps


_prog_cache = {}


def _get_program():
    if "nc" not in _prog_cache:
        nc = build_program()
        if not nc.is_finalized():
            nc.finalize()  # run Bacc passes (reg alloc, library loads)
        _prog_cache["nc"] = nc
    return _prog_cache["nc"]


def kernel(y_pred, y_true, perm, **run_kwargs):
    nc = _get_program()
    in_maps = make_in_maps(y_pred, y_true, perm)
    res = run_bass_kernel_spmd(
        nc, in_maps, core_ids=list(range(NCORES)), **run_kwargs
    )
    total = sum(float(r["out"][0, 0]) for r in res.results)
    out = np.float32(total / (B * S))
    if run_kwargs:
        return out, res
    return out


# revision 67
# speedup vs baseline: 1.0044x; 1.0044x over previous
"""MaxMargin loss kernel for 8 Trainium2 NeuronCores.

Reference computation (B=8192 rows, D=512, S=25 negative rounds):
    cos_pos[b]   = <y_true[b], y_pred[b]> / max(|y_true[b]||y_pred[b]|, eps)
    cos_neg[s,b] = <y_true[perm[s,b]], y_pred[b]> / max(|y_true[perm[s,b]]||y_pred[b]|, eps)
    out = mean_b( sum_s relu(1 - cos_pos + cos_neg) ) / S

Strategy: data-parallel over the batch dim (1024 rows of y_pred per
core); host casts inputs to bf16 (layout/precision prep only).

Prepass (software-pipelined): each core normalizes the full y_true into
an fp8-e4m3 row table in its DRAM.  Rows sit in (p n) layout so every
slab DMA is one contiguous 8KB chunk per partition; squares split
3 DVE / 5 ACT; normalize-copies run on DVE; slab l+1's squares are
emitted before slab l's copies so the rsqrt chain never stalls DVE, and
y_pred's copies are deferred past the last table write to overlap the
first gather's descriptor generation.

Gather phase: one single-packet dma_gather per round (26 rounds x 1024
rows x 512B, incl. the identity round 0 for cos_pos) from the fp8
table, 4 SWDGE queues round-robin.  The gather index order is permuted
host-side so gathered row i pairs with the (p n)-layout y_pred block.
Dots run as fused STT multiply+row-reduce ops on DVE (the measured
phase-3 floor: DVE is busy ~97% of the gather phase).  Margins run as
fused relu+sum activations on ACT; the final cross-partition sum is one
1-column matmul.  Host sums the 8 per-core partials.

Four tiny warmup gathers at prepass start absorb the first-use cost of
the SWDGE gather path while GPSIMD is idle; round 0 is split into four
256-row chunk gathers on separate queues so the first dots start ~6us
after the last table write instead of ~14us (a full 1024-row first gen
has ~9us completion latency); and cos_pos's (1 - cos) bias is computed
right after round 0 so the ACT margin ops overlap the final rounds'
dots.

Measured: ~213.4us on HW (baseline 310.2us, 1.45x), rel err ~1e-5 vs
the f32 reference (harness gate 2e-2).
"""

import os
import sys

import numpy as np

for _p in ("/opt/trn_rl_repo", "/root/.axon_site/_ro/trn_rl_repo"):
    if os.path.isdir(_p) and _p not in sys.path:
        sys.path.append(_p)

import ml_dtypes

import concourse.bacc as bacc
import concourse.bass as bass
import concourse.mybir as mybir
import concourse.tile as tile
from concourse.bass_utils import run_bass_kernel_spmd

B = 8192          # total batch rows
D = 512           # feature dim
S = 25            # negative-sampling rounds
NCORES = 8
SH = B // NCORES  # rows per core (1024)
NB = SH // 128    # 128-row blocks per core (8)
NSLAB = B // SH   # y_true slabs for the normalize pass (8)
NR = S + 1        # gather rounds incl. identity round 0 (26)
RB = 1            # rounds per dma_gather (single-packet)
GBUFS = 4         # gather tile buffering
NQ = 4            # swdge queues; round-robin gathers across them
F32 = mybir.dt.float32
BF16 = mybir.dt.bfloat16
I16 = mybir.dt.int16
FP8 = mybir.dt.float8e4

AX = mybir.AxisListType
ALU = mybir.AluOpType
ACTF = mybir.ActivationFunctionType


def build_program():
    nc = bacc.Bacc(None, target_bir_lowering=False, num_swdge_queues=NQ)

    yp = nc.dram_tensor("yp", [SH, D], BF16, kind="ExternalInput")
    yt = nc.dram_tensor("yt", [B, D], BF16, kind="ExternalInput")
    idx = nc.dram_tensor("idx", [128, NR * 64], I16, kind="ExternalInput")
    tt = nc.dram_tensor("tt", [B, D], FP8, kind="Internal")
    out = nc.dram_tensor("out", [1, 1], F32, kind="ExternalOutput")

    with tile.TileContext(nc) as tc:
        with (
            tc.tile_pool(name="singles", bufs=1) as singles,
            tc.tile_pool(name="slab", bufs=5) as slab_pool,
            tc.tile_pool(name="upool", bufs=5) as u_pool,
            tc.tile_pool(name="gpool", bufs=GBUFS) as g_pool,
            tc.tile_pool(name="scr", bufs=6) as scr_pool,
            tc.tile_pool(name="small", bufs=6) as small_pool,
            tc.tile_pool(name="psum", bufs=1, space="PSUM") as psum_pool,
        ):
            idx_sb = singles.tile([128, NR * 64], I16)

            zero_b = singles.tile([128, 1], F32)
            nc.vector.memset(zero_b, 0.0)

            def dot_stt(in0, in1, accum_out):
                """fused multiply + row-reduce via STT; one DVE pass (1x)."""
                scr = scr_pool.tile([128, D], BF16, tag="dot_scr")
                nc.vector.scalar_tensor_tensor(
                    out=scr,
                    in0=in0,
                    scalar=1.0,
                    in1=in1,
                    op0=ALU.mult,
                    op1=ALU.mult,
                    accum_out=accum_out,
                )

            def dot_act_sq(in0, in1, accum_out):
                """DVE bf16 add (2x) + ACT square-accum on the idle engine.

                accum_out = ||in0 + in1||^2 = 2 + 2*cos for unit rows;
                those cn columns are rescaled to cos before the margins."""
                scr = scr_pool.tile([128, D], BF16, tag="dot_scr")
                nc.vector.tensor_tensor(
                    out=scr, in0=in0, in1=in1, op=ALU.add
                )
                act_scr = scr_pool.tile([128, D], BF16, tag="act_scr")
                nc.scalar.activation(
                    out=act_scr,
                    in_=scr,
                    func=ACTF.Square,
                    bias=0.0,
                    scale=1.0,
                    accum_out=accum_out,
                )

            def square_pass(x):
                """rowwise 1/|row| for a [128, NB, D] bf16 slab -> [128, NB].

                Squares split 3 DVE / 5 ACT (ACT's accumulator read makes
                its squares ~1.4x a DVE STT)."""
                ssq = small_pool.tile([128, NB], F32, tag="ssq")
                for n in range(NB):
                    if n % 3 == 0:
                        dot_stt(x[:, n, :], x[:, n, :], ssq[:, n : n + 1])
                    else:
                        act_scr = scr_pool.tile([128, D], BF16, tag="act_scr")
                        nc.scalar.activation(
                            out=act_scr,
                            in_=x[:, n, :],
                            func=ACTF.Square,
                            bias=0.0,
                            scale=1.0,
                            accum_out=ssq[:, n : n + 1],
                        )
                ssqm = small_pool.tile([128, NB], F32, tag="ssqm")
                nc.vector.tensor_scalar_max(out=ssqm, in0=ssq, scalar1=1e-30)
                inv = small_pool.tile([128, NB], F32, tag="inv")
                nc.vector.reciprocal(out=inv, in_=ssqm)
                rs = small_pool.tile([128, NB], F32, tag="rs")
                nc.scalar.activation(
                    out=rs, in_=inv, func=ACTF.Sqrt, bias=zero_b, scale=1.0
                )
                return rs

            def copy_pass(x, rs, u):
                for n in range(NB):
                    nc.vector.tensor_scalar_mul(
                        out=u[:, n, :], in0=x[:, n, :], scalar1=rs[:, n : n + 1]
                    )

            # ---- software-pipelined prepass ------------------------------
            # Emit slab l+1's squares before slab l's copies so DVE never
            # stalls on the rsqrt chain; y_pred's copies are deferred to the
            # end, overlapping the first gather's descriptor generation.
            up = singles.tile([128, NB, D], BF16)
            xp = singles.tile([128, NB, D], BF16)
            ypv = yp[:, :].rearrange("(p n) d -> p n d", n=NB)
            nc.sync.dma_start(out=xp[:, : NB // 2, :], in_=ypv[:, : NB // 2, :])
            nc.sync.dma_start(out=xp[:, NB // 2 :, :], in_=ypv[:, NB // 2 :, :])
            nc.sync.dma_start(out=idx_sb, in_=idx[:, :])

            # Warm up the SWDGE gather machinery (one-time pool-config/ring
            # init costs ~9us) with tiny 16-row gathers on each queue while
            # GPSIMD is otherwise idle; sources yt, which is ready at t=0.
            ytw = yt[:, :].rearrange("b (c e) -> (b c) e", e=128)
            for q in range(NQ):
                warm = scr_pool.tile([128, 1, 128], BF16, tag="warm")
                nc.gpsimd.dma_gather(
                    warm[:, :, :],
                    ytw,
                    idx_sb[:, 0:1],
                    num_idxs=16,
                    num_idxs_reg=16,
                    elem_size=128,
                    single_packet=True,
                    queue_num=q,
                )
            rs_p = square_pass(xp)
            rs_p_keep = singles.tile([128, NB], F32)
            nc.vector.tensor_copy(out=rs_p_keep, in_=rs_p)

            pend = None  # (x, rs, u, l) awaiting copies+writeback
            for l in range(NSLAB):
                x = slab_pool.tile([128, NB, D], BF16, tag="x")
                nc.sync.dma_start(
                    out=x,
                    in_=yt[l * SH : (l + 1) * SH, :].rearrange(
                        "(p n) d -> p n d", n=NB
                    ),
                )
                rs = square_pass(x)
                if pend is not None:
                    px, prs, pu, pl = pend
                    copy_pass(px, prs, pu)
                    nc.sync.dma_start(
                        out=tt[pl * SH : (pl + 1) * SH, :].rearrange(
                            "(p n) d -> p n d", n=NB
                        ),
                        in_=pu,
                    )
                u = u_pool.tile([128, NB, D], FP8, tag="u")
                pend = (x, rs, u, l)
            px, prs, pu, pl = pend
            ttv = tt[pl * SH : (pl + 1) * SH, :].rearrange(
                "(p n) d -> p n d", n=NB
            )
            h = NB // 2
            for n in range(h):
                nc.vector.tensor_scalar_mul(
                    out=pu[:, n, :], in0=px[:, n, :], scalar1=prs[:, n : n + 1]
                )
            nc.sync.dma_start(out=ttv[:, :h, :], in_=pu[:, :h, :])
            for n in range(h, NB):
                nc.vector.tensor_scalar_mul(
                    out=pu[:, n, :], in0=px[:, n, :], scalar1=prs[:, n : n + 1]
                )
            nc.sync.dma_start(out=ttv[:, h:, :], in_=pu[:, h:, :])

            # late warmup: a dummy gather anchored (via a data dep on the
            # last slab's rsqrt) to run right before the first real gather,
            # absorbing the ~9us first-gen-after-idle SWDGE penalty while
            # the final table writes drain.  All-zero indices gather row 0.
            zf = small_pool.tile([128, 1], F32, tag="zf")
            nc.vector.tensor_scalar_mul(out=zf, in0=prs[:, 0:1], scalar1=0.0)
            idx_dummy = small_pool.tile([128, 1], I16, tag="idxd")
            nc.vector.tensor_copy(out=idx_dummy, in_=zf)
            warm_l = scr_pool.tile([128, 1, 128], BF16, tag="warm")
            nc.gpsimd.dma_gather(
                warm_l[:, :, :],
                ytw,
                idx_dummy[:, 0:1],
                num_idxs=16,
                num_idxs_reg=16,
                elem_size=128,
                single_packet=True,
                queue_num=0,
            )
            copy_pass(xp, rs_p_keep, up)

            # ---- gather rounds + fused dot products ----
            # CN[:, n, r] = cos of round r for row block n (round 0 = cos_pos)
            # rounds >= V3_START hold 2 + 2*cos instead (variant 3).
            cn = singles.tile([128, NB, NR], F32)
            nc.vector.memset(cn, 0.0)
            cpb = singles.tile([128, NB], F32)  # 1 - cos_pos
            # Ramp: a full 1024-row gen takes ~90ns to post but ~9us to
            # complete, so rounds 2-5's gens are posted FIRST (completing
            # concurrently while DVE works), and rounds 0-1 are split into
            # 256-row chunk gathers whose short completion feeds the dots
            # immediately after the last table write.
            PRE = 4  # gen prefetch depth (= GBUFS)

            def full_gather(s, g):
                nc.gpsimd.dma_gather(
                    g[:, :, :],
                    tt[:, :],
                    idx_sb[:, s * 64 : (s + 1) * 64],
                    num_idxs=SH,
                    num_idxs_reg=SH,
                    elem_size=D,
                    single_packet=(SH // 16) <= 64,
                    queue_num=s % NQ,
                )

            gtiles = {}
            for s in range(2, 2 + PRE):
                g = g_pool.tile([128, NB, D], FP8, tag="g")
                full_gather(s, g)
                gtiles[s] = g
            for s in (0, 1):
                for c in range(4):
                    gc = scr_pool.tile([128, 2, D], FP8, tag="g0")
                    nc.gpsimd.dma_gather(
                        gc[:, :, :],
                        tt[:, :],
                        idx_sb[:, s * 64 + c * 16 : s * 64 + (c + 1) * 16],
                        num_idxs=SH // 4,
                        num_idxs_reg=SH // 4,
                        elem_size=D,
                        single_packet=True,
                        queue_num=c,
                    )
                    for k in range(2):
                        n = 2 * c + k
                        dot_stt(gc[:, k, :], up[:, n, :], cn[:, n, s : s + 1])
                if s == 0:
                    # cpb needs only round 0; computing it here lets the
                    # ACT margin ops overlap the final rounds' dots.
                    nc.vector.tensor_scalar(
                        out=cpb,
                        in0=cn[:, :, 0],
                        scalar1=-1.0,
                        scalar2=1.0,
                        op0=ALU.mult,
                        op1=ALU.add,
                    )
            for s in range(2, NR):
                g = gtiles.pop(s)
                for n in range(NB):
                    dot_stt(g[:, n, :], up[:, n, :], cn[:, n, s : s + 1])
                if s + PRE < NR:
                    gnext = g_pool.tile([128, NB, D], FP8, tag="g")
                    full_gather(s + PRE, gnext)
                    gtiles[s + PRE] = gnext

            # ---- margins: sum_s relu((1 - cos_pos) + cos_neg) ----
            # margin+sum fused on ACT: mt[:, n] = sum_s relu(cn + cpb)
            mt = singles.tile([128, NB], F32)
            for n in range(NB):
                m_scr = scr_pool.tile([128, S], F32, tag="m_scr")
                nc.scalar.activation(
                    out=m_scr,
                    in_=cn[:, n, 1:NR],
                    func=ACTF.Relu,
                    bias=cpb[:, n : n + 1],
                    scale=1.0,
                    accum_out=mt[:, n : n + 1],
                )

            # ---- partial = sum over partitions and blocks ----
            mts = singles.tile([128, 1], F32)
            nc.vector.reduce_sum(out=mts, in_=mt, axis=AX.X)
            ones = singles.tile([128, 1], F32)
            nc.vector.memset(ones, 1.0)
            ps = psum_pool.tile([1, 1], F32)
            nc.tensor.matmul(ps, ones, mts, start=True, stop=True)
            osb = singles.tile([1, 1], F32)
            nc.vector.tensor_copy(out=osb, in_=ps)
            nc.sync.dma_start(out=out[:, :], in_=osb)

    return nc


def make_in_maps(y_pred, y_true, perm):
    """Shard the full inputs into the 8 per-core input maps."""
    y_pred = np.ascontiguousarray(y_pred, dtype=np.float32).astype(
        ml_dtypes.bfloat16
    )
    y_true = np.ascontiguousarray(y_true, dtype=np.float32).astype(
        ml_dtypes.bfloat16
    )
    perm = np.asarray(perm)
    in_maps = []
    for c in range(NCORES):
        ident = (c * SH + np.arange(SH, dtype=np.int64))[None, :]
        rounds = np.concatenate(
            [ident, perm[:, c * SH : (c + 1) * SH].astype(np.int64)], axis=0
        )  # [NR, SH]
        # dma_gather index layout: flat index i lives at partition i%16,
        # free slot i//16, replicated across the 8 groups of 16 partitions.
        # g row at gather position i lands at [i%128, i//128]; up (p n)
        # layout puts batch row b at [b//NB, b%NB] -> remap i = (b%NB)*128+b//NB
        i_of = np.arange(SH)
        remap = (i_of % 128) * NB + i_of // 128  # b gathered at position i
        rounds = rounds[:, remap]
        w = rounds.reshape(NR, SH // 16, 16).transpose(0, 2, 1)  # [NR,16,64]
        rep = np.broadcast_to(w[:, None, :, :], (NR, 8, 16, SH // 16))
        idx = (
            rep.reshape(NR, 128, SH // 16)
            .transpose(1, 0, 2)
            .reshape(128, NR * (SH // 16))
            .astype(np.int16)
        )
        in_maps.append(
            {
                "yp": np.ascontiguousarray(y_pred[c * SH : (c + 1) * SH]),
                "yt": y_true,
                "idx": np.ascontiguousarray(idx),
            }
        )
    return in_maps


_prog_cache = {}


def _get_program():
    if "nc" not in _prog_cache:
        nc = build_program()
        if not nc.is_finalized():
            nc.finalize()  # run Bacc passes (reg alloc, library loads)
        _prog_cache["nc"] = nc
    return _prog_cache["nc"]


def kernel(y_pred, y_true, perm, **run_kwargs):
    nc = _get_program()
    in_maps = make_in_maps(y_pred, y_true, perm)
    res = run_bass_kernel_spmd(
        nc, in_maps, core_ids=list(range(NCORES)), **run_kwargs
    )
    total = sum(float(r["out"][0, 0]) for r in res.results)
    out = np.float32(total / (B * S))
    if run_kwargs:
        return out, res
    return out


# revision 68
# speedup vs baseline: 1.0442x; 1.0396x over previous
"""MaxMargin loss kernel for 8 Trainium2 NeuronCores.

Reference computation (B=8192 rows, D=512, S=25 negative rounds):
    cos_pos[b]   = <y_true[b], y_pred[b]> / max(|y_true[b]||y_pred[b]|, eps)
    cos_neg[s,b] = <y_true[perm[s,b]], y_pred[b]> / max(|y_true[perm[s,b]]||y_pred[b]|, eps)
    out = mean_b( sum_s relu(1 - cos_pos + cos_neg) ) / S

Strategy: data-parallel over the batch dim (1024 rows of y_pred per
core); host casts inputs to bf16 (layout/precision prep only).

Prepass (software-pipelined): each core normalizes the full y_true into
an fp8-e4m3 row table in its DRAM.  Rows sit in (p n) layout so every
slab DMA is one contiguous 8KB chunk per partition; squares split
3 DVE / 5 ACT; normalize-copies run on DVE; slab l+1's squares are
emitted before slab l's copies so the rsqrt chain never stalls DVE, and
y_pred's copies are deferred past the last table write to overlap the
first gather's descriptor generation.

Gather phase: one single-packet dma_gather per round (26 rounds x 1024
rows x 512B, incl. the identity round 0 for cos_pos) from the fp8
table, 4 SWDGE queues round-robin.  The gather index order is permuted
host-side so gathered row i pairs with the (p n)-layout y_pred block.
Dots run as fused STT multiply+row-reduce ops on DVE (the measured
phase-3 floor: DVE is busy ~97% of the gather phase).  Margins run as
fused relu+sum activations on ACT; the final cross-partition sum is one
1-column matmul.  Host sums the 8 per-core partials.

Four tiny warmup gathers at prepass start absorb the first-use cost of
the SWDGE gather path while GPSIMD is idle; round 0 is split into four
256-row chunk gathers on separate queues so the first dots start ~6us
after the last table write instead of ~14us (a full 1024-row first gen
has ~9us completion latency); and cos_pos's (1 - cos) bias is computed
right after round 0 so the ACT margin ops overlap the final rounds'
dots.

Measured: ~213.4us on HW (baseline 310.2us, 1.45x), rel err ~1e-5 vs
the f32 reference (harness gate 2e-2).
"""

import os
import sys

import numpy as np

for _p in ("/opt/trn_rl_repo", "/root/.axon_site/_ro/trn_rl_repo"):
    if os.path.isdir(_p) and _p not in sys.path:
        sys.path.append(_p)

import ml_dtypes

import concourse.bacc as bacc
import concourse.bass as bass
import concourse.mybir as mybir
import concourse.tile as tile
from concourse.bass_utils import run_bass_kernel_spmd

B = 8192          # total batch rows
D = 512           # feature dim
S = 25            # negative-sampling rounds
NCORES = 8
SH = B // NCORES  # rows per core (1024)
NB = SH // 128    # 128-row blocks per core (8)
NSLAB = B // SH   # y_true slabs for the normalize pass (8)
NR = S + 1        # gather rounds incl. identity round 0 (26)
RB = 1            # rounds per dma_gather (single-packet)
GBUFS = 4         # gather tile buffering
NQ = 4            # swdge queues; round-robin gathers across them
F32 = mybir.dt.float32
BF16 = mybir.dt.bfloat16
I16 = mybir.dt.int16
FP8 = mybir.dt.float8e4

AX = mybir.AxisListType
ALU = mybir.AluOpType
ACTF = mybir.ActivationFunctionType


def build_program():
    nc = bacc.Bacc(None, target_bir_lowering=False, num_swdge_queues=NQ)

    yp = nc.dram_tensor("yp", [SH, D], BF16, kind="ExternalInput")
    yt = nc.dram_tensor("yt", [B, D], BF16, kind="ExternalInput")
    idx = nc.dram_tensor("idx", [128, NR * 64], I16, kind="ExternalInput")
    tt = nc.dram_tensor("tt", [B, D], FP8, kind="Internal")
    out = nc.dram_tensor("out", [1, 1], F32, kind="ExternalOutput")

    with tile.TileContext(nc) as tc:
        with (
            tc.tile_pool(name="singles", bufs=1) as singles,
            tc.tile_pool(name="slab", bufs=6) as slab_pool,
            tc.tile_pool(name="upool", bufs=5) as u_pool,
            tc.tile_pool(name="gpool", bufs=GBUFS) as g_pool,
            tc.tile_pool(name="scr", bufs=6) as scr_pool,
            tc.tile_pool(name="small", bufs=6) as small_pool,
            tc.tile_pool(name="psum", bufs=1, space="PSUM") as psum_pool,
        ):
            idx_sb = singles.tile([128, NR * 64], I16)

            zero_b = singles.tile([128, 1], F32)
            nc.vector.memset(zero_b, 0.0)

            def dot_stt(in0, in1, accum_out):
                """fused multiply + row-reduce via STT; one DVE pass (1x)."""
                scr = scr_pool.tile([128, D], BF16, tag="dot_scr")
                nc.vector.scalar_tensor_tensor(
                    out=scr,
                    in0=in0,
                    scalar=1.0,
                    in1=in1,
                    op0=ALU.mult,
                    op1=ALU.mult,
                    accum_out=accum_out,
                )

            def dot_act_sq(in0, in1, accum_out):
                """DVE bf16 add (2x) + ACT square-accum on the idle engine.

                accum_out = ||in0 + in1||^2 = 2 + 2*cos for unit rows;
                those cn columns are rescaled to cos before the margins."""
                scr = scr_pool.tile([128, D], BF16, tag="dot_scr")
                nc.vector.tensor_tensor(
                    out=scr, in0=in0, in1=in1, op=ALU.add
                )
                act_scr = scr_pool.tile([128, D], BF16, tag="act_scr")
                nc.scalar.activation(
                    out=act_scr,
                    in_=scr,
                    func=ACTF.Square,
                    bias=0.0,
                    scale=1.0,
                    accum_out=accum_out,
                )

            def square_pass(x):
                """rowwise 1/|row| for a [128, NB, D] bf16 slab -> [128, NB].

                Squares split 3 DVE / 5 ACT (ACT's accumulator read makes
                its squares ~1.4x a DVE STT)."""
                ssq = small_pool.tile([128, NB], F32, tag="ssq")
                for n in range(NB):
                    if n % 3 == 0:
                        dot_stt(x[:, n, :], x[:, n, :], ssq[:, n : n + 1])
                    else:
                        act_scr = scr_pool.tile([128, D], BF16, tag="act_scr")
                        nc.scalar.activation(
                            out=act_scr,
                            in_=x[:, n, :],
                            func=ACTF.Square,
                            bias=0.0,
                            scale=1.0,
                            accum_out=ssq[:, n : n + 1],
                        )
                ssqm = small_pool.tile([128, NB], F32, tag="ssqm")
                nc.vector.tensor_scalar_max(out=ssqm, in0=ssq, scalar1=1e-30)
                inv = small_pool.tile([128, NB], F32, tag="inv")
                nc.vector.reciprocal(out=inv, in_=ssqm)
                rs = small_pool.tile([128, NB], F32, tag="rs")
                nc.scalar.activation(
                    out=rs, in_=inv, func=ACTF.Sqrt, bias=zero_b, scale=1.0
                )
                return rs

            def copy_pass(x, rs, u):
                for n in range(NB):
                    nc.vector.tensor_scalar_mul(
                        out=u[:, n, :], in0=x[:, n, :], scalar1=rs[:, n : n + 1]
                    )

            # ---- software-pipelined prepass ------------------------------
            # Emit slab l+1's squares before slab l's copies so DVE never
            # stalls on the rsqrt chain; y_pred's copies are deferred to the
            # end, overlapping the first gather's descriptor generation.
            up = singles.tile([128, NB, D], BF16)
            xp = singles.tile([128, NB, D], BF16)
            ypv = yp[:, :].rearrange("(p n) d -> p n d", n=NB)
            nc.sync.dma_start(out=xp[:, : NB // 2, :], in_=ypv[:, : NB // 2, :])
            nc.sync.dma_start(out=xp[:, NB // 2 :, :], in_=ypv[:, NB // 2 :, :])
            nc.sync.dma_start(out=idx_sb, in_=idx[:, :])

            # Warm up the SWDGE gather machinery (one-time pool-config/ring
            # init costs ~9us) with tiny 16-row gathers on each queue while
            # GPSIMD is otherwise idle; sources yt, which is ready at t=0.
            ytw = yt[:, :].rearrange("b (c e) -> (b c) e", e=128)
            for q in range(NQ):
                warm = scr_pool.tile([128, 1, 128], BF16, tag="warm")
                nc.gpsimd.dma_gather(
                    warm[:, :, :],
                    ytw,
                    idx_sb[:, 0:1],
                    num_idxs=16,
                    num_idxs_reg=16,
                    elem_size=128,
                    single_packet=True,
                    queue_num=q,
                )
            rs_p = square_pass(xp)
            rs_p_keep = singles.tile([128, NB], F32)
            nc.vector.tensor_copy(out=rs_p_keep, in_=rs_p)

            pend = None  # (x, rs, u, l) awaiting copies+writeback
            for l in range(NSLAB):
                x = slab_pool.tile([128, NB, D], BF16, tag="x")
                nc.sync.dma_start(
                    out=x,
                    in_=yt[l * SH : (l + 1) * SH, :].rearrange(
                        "(p n) d -> p n d", n=NB
                    ),
                )
                rs = square_pass(x)
                if pend is not None:
                    px, prs, pu, pl = pend
                    copy_pass(px, prs, pu)
                    nc.sync.dma_start(
                        out=tt[pl * SH : (pl + 1) * SH, :].rearrange(
                            "(p n) d -> p n d", n=NB
                        ),
                        in_=pu,
                    )
                u = u_pool.tile([128, NB, D], FP8, tag="u")
                pend = (x, rs, u, l)
            px, prs, pu, pl = pend
            ttv = tt[pl * SH : (pl + 1) * SH, :].rearrange(
                "(p n) d -> p n d", n=NB
            )
            h = NB // 2
            for n in range(h):
                nc.vector.tensor_scalar_mul(
                    out=pu[:, n, :], in0=px[:, n, :], scalar1=prs[:, n : n + 1]
                )
            nc.sync.dma_start(out=ttv[:, :h, :], in_=pu[:, :h, :])
            for n in range(h, NB):
                nc.vector.tensor_scalar_mul(
                    out=pu[:, n, :], in0=px[:, n, :], scalar1=prs[:, n : n + 1]
                )
            nc.sync.dma_start(out=ttv[:, h:, :], in_=pu[:, h:, :])

            # late warmup: a dummy gather anchored (via a data dep on the
            # last slab's rsqrt) to run right before the first real gather,
            # absorbing the ~9us first-gen-after-idle SWDGE penalty while
            # the final table writes drain.  All-zero indices gather row 0.
            zf = small_pool.tile([128, 1], F32, tag="zf")
            nc.vector.tensor_scalar_mul(out=zf, in0=prs[:, 0:1], scalar1=0.0)
            idx_dummy = small_pool.tile([128, 1], I16, tag="idxd")
            nc.vector.tensor_copy(out=idx_dummy, in_=zf)
            warm_l = scr_pool.tile([128, 1, 128], BF16, tag="warm")
            nc.gpsimd.dma_gather(
                warm_l[:, :, :],
                ytw,
                idx_dummy[:, 0:1],
                num_idxs=16,
                num_idxs_reg=16,
                elem_size=128,
                single_packet=True,
                queue_num=0,
            )
            copy_pass(xp, rs_p_keep, up)

            # ---- gather rounds + fused dot products ----
            # CN[:, n, r] = cos of round r for row block n (round 0 = cos_pos)
            # rounds >= V3_START hold 2 + 2*cos instead (variant 3).
            cn = singles.tile([128, NB, NR], F32)
            nc.vector.memset(cn, 0.0)
            cpb = singles.tile([128, NB], F32)  # 1 - cos_pos
            # Round 0 is split into 4 small gathers on separate queues: the
            # first 1024-row gen after the prepass costs ~8.6us (cold SWDGE
            # path), gating phase-3 start; 256-row chunks start the dots
            # several us earlier and per-chunk tiles keep the deps precise.
            for c in range(4):
                gc = scr_pool.tile([128, 2, D], FP8, tag="g0")
                nc.gpsimd.dma_gather(
                    gc[:, :, :],
                    tt[:, :],
                    idx_sb[:, c * 16 : (c + 1) * 16],
                    num_idxs=SH // 4,
                    num_idxs_reg=SH // 4,
                    elem_size=D,
                    single_packet=True,
                    queue_num=c,
                )
                for k in range(2):
                    n = 2 * c + k
                    dot_stt(gc[:, k, :], up[:, n, :], cn[:, n, 0:1])
            for s in range(1, NR):
                g = g_pool.tile([128, NB, D], FP8, tag="g")
                nc.gpsimd.dma_gather(
                    g[:, :, :],
                    tt[:, :],
                    idx_sb[:, s * 64 : (s + 1) * 64],
                    num_idxs=SH,
                    num_idxs_reg=SH,
                    elem_size=D,
                    single_packet=(SH // 16) <= 64,
                    queue_num=s % NQ,
                )
                for n in range(NB):
                    dot_stt(g[:, n, :], up[:, n, :], cn[:, n, s : s + 1])
                if s == 1:
                    # cpb needs only round 0; computing it here lets the
                    # ACT margin ops overlap the final rounds' dots.
                    nc.vector.tensor_scalar(
                        out=cpb,
                        in0=cn[:, :, 0],
                        scalar1=-1.0,
                        scalar2=1.0,
                        op0=ALU.mult,
                        op1=ALU.add,
                    )

            # ---- margins: sum_s relu((1 - cos_pos) + cos_neg) ----
            # margin+sum fused on ACT: mt[:, n] = sum_s relu(cn + cpb)
            mt = singles.tile([128, NB], F32)
            for n in range(NB):
                m_scr = scr_pool.tile([128, S], F32, tag="m_scr")
                nc.scalar.activation(
                    out=m_scr,
                    in_=cn[:, n, 1:NR],
                    func=ACTF.Relu,
                    bias=cpb[:, n : n + 1],
                    scale=1.0,
                    accum_out=mt[:, n : n + 1],
                )

            # ---- partial = sum over partitions and blocks ----
            mts = singles.tile([128, 1], F32)
            nc.vector.reduce_sum(out=mts, in_=mt, axis=AX.X)
            ones = singles.tile([128, 1], F32)
            nc.vector.memset(ones, 1.0)
            ps = psum_pool.tile([1, 1], F32)
            nc.tensor.matmul(ps, ones, mts, start=True, stop=True)
            osb = singles.tile([1, 1], F32)
            nc.vector.tensor_copy(out=osb, in_=ps)
            nc.sync.dma_start(out=out[:, :], in_=osb)

    return nc


def make_in_maps(y_pred, y_true, perm):
    """Shard the full inputs into the 8 per-core input maps."""
    y_pred = np.ascontiguousarray(y_pred, dtype=np.float32).astype(
        ml_dtypes.bfloat16
    )
    y_true = np.ascontiguousarray(y_true, dtype=np.float32).astype(
        ml_dtypes.bfloat16
    )
    perm = np.asarray(perm)
    in_maps = []
    for c in range(NCORES):
        ident = (c * SH + np.arange(SH, dtype=np.int64))[None, :]
        rounds = np.concatenate(
            [ident, perm[:, c * SH : (c + 1) * SH].astype(np.int64)], axis=0
        )  # [NR, SH]
        # dma_gather index layout: flat index i lives at partition i%16,
        # free slot i//16, replicated across the 8 groups of 16 partitions.
        # g row at gather position i lands at [i%128, i//128]; up (p n)
        # layout puts batch row b at [b//NB, b%NB] -> remap i = (b%NB)*128+b//NB
        i_of = np.arange(SH)
        remap = (i_of % 128) * NB + i_of // 128  # b gathered at position i
        rounds = rounds[:, remap]
        w = rounds.reshape(NR, SH // 16, 16).transpose(0, 2, 1)  # [NR,16,64]
        rep = np.broadcast_to(w[:, None, :, :], (NR, 8, 16, SH // 16))
        idx = (
            rep.reshape(NR, 128, SH // 16)
            .transpose(1, 0, 2)
            .reshape(128, NR * (SH // 16))
            .astype(np.int16)
        )
        in_maps.append(
            {
                "yp": np.ascontiguousarray(y_pred[c * SH : (c + 1) * SH]),
                "yt": y_true,
                "idx": np.ascontiguousarray(idx),
            }
        )
    return in_maps


_prog_cache = {}


def _get_program():
    if "nc" not in _prog_cache:
        nc = build_program()
        if not nc.is_finalized():
            nc.finalize()  # run Bacc passes (reg alloc, library loads)
        _prog_cache["nc"] = nc
    return _prog_cache["nc"]


def kernel(y_pred, y_true, perm, **run_kwargs):
    nc = _get_program()
    in_maps = make_in_maps(y_pred, y_true, perm)
    res = run_bass_kernel_spmd(
        nc, in_maps, core_ids=list(range(NCORES)), **run_kwargs
    )
    total = sum(float(r["out"][0, 0]) for r in res.results)
    out = np.float32(total / (B * S))
    if run_kwargs:
        return out, res
    return out
